# revision 4
# baseline (speedup 1.0000x reference)
"""Trainium2 Bass kernel for sparse-attention aspect pooling.

reference math (per batch row b):
    proj[a,l,h]  = sum_d x[l,d] * P[a,d,h]
    score[a,l]   = sum_{w,h} proj[a,l+w-1,h] * E[a,w,h]   (zero-padded window)
    attn[a,l]    = softmax_l(score)
    rep[a,h]     = sum_l attn[a,l] * proj[a,l,h]

Sharding: pure data parallel over batch (512 -> 64 per core x 8 cores).
Each core processes 2 batch rows per iteration:
  - proj via col-tiled matmuls: psum[0:64]=b0, psum[64:128]=b1, rows c=(h*5+a)
  - score via 3 shifted matmuls against a block-diagonal selector built from E;
    output rows replicated across h so pooling is a row-wise mult+reduce
  - softmax needs no max-subtraction (|score| < ~0.05 for this problem's scale)
"""

import numpy as np
import ml_dtypes

import concourse.bass as bass
import concourse.mybir as mybir
import concourse.tile as tile
from concourse import bacc
from concourse.bass_utils import run_bass_kernel_spmd

BF16 = mybir.dt.bfloat16
F32 = mybir.dt.float32

N_CORES = 8
BATCH = 512
B_CORE = BATCH // N_CORES      # 64
L = 500
D = 300
A = 5
H = 10
CTX = 3
C = A * H                      # 50 used rows, padded to 64 per batch row
PAIRS = B_CORE // 2            # 32 iterations, 2 batch rows each

_CACHE = {}


def _build():
    nc = bacc.Bacc(
        "TRN2", target_bir_lowering=False, debug=False, num_devices=N_CORES
    )
    xt_h = nc.dram_tensor("xt", [D, B_CORE * L], BF16, kind="ExternalInput")
    wm_h = nc.dram_tensor("wm", [D, 64], BF16, kind="ExternalInput")
    sel_h = nc.dram_tensor("sel", [128, CTX, 128], BF16, kind="ExternalInput")
    attn_h = nc.dram_tensor("attn_o", [B_CORE, A, L], F32, kind="ExternalOutput")
    rep_h = nc.dram_tensor("rep_o", [128, PAIRS], F32, kind="ExternalOutput")

    xt = xt_h.ap()
    wm = wm_h.ap()
    sel = sel_h.ap()
    attn_o = attn_h.ap()
    rep_o = rep_h.ap()

    from contextlib import ExitStack

    with tile.TileContext(nc) as tc, ExitStack() as ctx:
        singles = ctx.enter_context(tc.tile_pool(name="singles", bufs=1))
        xin = ctx.enter_context(tc.tile_pool(name="xin", bufs=3))
        psum = ctx.enter_context(tc.tile_pool(name="psum", bufs=2, space="PSUM"))
        work = ctx.enter_context(tc.tile_pool(name="work", bufs=3))
        small = ctx.enter_context(tc.tile_pool(name="small", bufs=4))

        wt = singles.tile([128, 3, 64], BF16)
        nc.sync.dma_start(out=wt[:, 0, :], in_=wm[0:128, :])
        nc.sync.dma_start(out=wt[:, 1, :], in_=wm[128:256, :])
        nc.sync.dma_start(out=wt[0:44, 2, :], in_=wm[256:300, :])
        selt = singles.tile([128, CTX, 128], BF16)
        nc.sync.dma_start(out=selt[:, :, :], in_=sel[:, :, :])
        repS = singles.tile([128, PAIRS], F32)

        kchunks = ((0, 128), (1, 128), (2, 44))

        for i in range(PAIRS):
            c0 = i * 2 * L
            xa = xin.tile([128, 2 * L], BF16, tag="xa")
            xb = xin.tile([128, 2 * L], BF16, tag="xb")
            xc = xin.tile([128, 2 * L], BF16, tag="xc")
            nc.sync.dma_start(out=xa[:, :], in_=xt[0:128, c0 : c0 + 2 * L])
            nc.sync.dma_start(out=xb[:, :], in_=xt[128:256, c0 : c0 + 2 * L])
            nc.sync.dma_start(out=xc[0:44, :], in_=xt[256:300, c0 : c0 + 2 * L])
            xk_tiles = (xa, xb, xc)

            # proj: psum rows [0:64] = batch 2i, rows [64:128] = batch 2i+1
            pP = psum.tile([128, 512], F32, tag="pP")
            for half in (0, 1):
                for k, krows in kchunks:
                    xk = xk_tiles[k]
                    nc.tensor.matmul(
                        pP[64 * half : 64 * half + 64, 0:L],
                        wt[0:krows, k, :],
                        xk[0:krows, half * L : half * L + L],
                        start=(k == 0),
                        stop=(k == 2),
                    )

            sbP = work.tile([128, L], BF16, tag="sbP")
            nc.scalar.copy(out=sbP[:, :], in_=pP[:, 0:L])

            # score: 3 shifted selector matmuls accumulate into pS
            pS = psum.tile([128, 512], F32, tag="pS")
            nc.tensor.matmul(
                pS[:, 0:L], selt[:, 1, :], sbP[:, 0:L], start=True, stop=False
            )
            nc.tensor.matmul(
                pS[:, 1:L], selt[:, 0, :], sbP[:, 0 : L - 1], start=False, stop=False
            )
            nc.tensor.matmul(
                pS[:, 0 : L - 1], selt[:, 2, :], sbP[:, 1:L], start=False, stop=True
            )

            sbE = work.tile([128, L], BF16, tag="sbE")
            den = small.tile([128, 1], F32, tag="den")
            nc.scalar.activation(
                out=sbE[:, :],
                in_=pS[:, 0:L],
                func=mybir.ActivationFunctionType.Exp,
                accum_out=den[:, :],
            )
            invden = small.tile([128, 1], F32, tag="invden")
            nc.vector.reciprocal(out=invden[:, :], in_=den[:, :])

            # rep[:, i] = sum_l (sbE * invden) * sbP  — attn-weighted pooling
            trash = work.tile([128, L], BF16, tag="trash")
            nc.vector.scalar_tensor_tensor(
                out=trash[:, :],
                in0=sbE[:, :],
                scalar=invden[:, :],
                in1=sbP[:, :],
                op0=mybir.AluOpType.mult,
                op1=mybir.AluOpType.mult,
                accum_out=repS[:, i : i + 1],
            )

            attnS = work.tile([128, L], F32, tag="attnS")
            nc.vector.tensor_scalar_mul(attnS[:, :], sbE[:, :], invden[:, :])

            nc.sync.dma_start(out=attn_o[2 * i, :, :], in_=attnS[0:A, :])
            nc.sync.dma_start(out=attn_o[2 * i + 1, :, :], in_=attnS[64 : 64 + A, :])

        nc.sync.dma_start(out=rep_o[:, :], in_=repS[:, :])

    nc.compile()
    return nc


def _get_nc():
    if "nc" not in _CACHE:
        _CACHE["nc"] = _build()
    return _CACHE["nc"]


def _prep_inputs(review_emb, asp_embed, asp_proj):
    """Host-side shard + layout prep. Returns in_maps for run_bass_kernel_spmd."""
    x = np.asarray(review_emb, dtype=np.float32)
    E = np.asarray(asp_embed, dtype=np.float32).reshape(A, CTX, H)
    P = np.asarray(asp_proj, dtype=np.float32)

    # W[d, h*5+a] = P[a,d,h], padded to 64 cols
    wm = np.zeros((D, 64), dtype=np.float32)
    wm[:, :C] = P.transpose(1, 2, 0).reshape(D, C)
    wm = wm.astype(ml_dtypes.bfloat16)

    # selector: S_w[(h'*5+a'), (h*5+a)] = delta(a'==a) * E[a,w,h']
    # block-diagonal duplicate for the two stacked batch rows
    sel = np.zeros((128, CTX, 128), dtype=np.float32)
    hh = np.arange(H)
    for w in range(CTX):
        S = np.zeros((64, 64), dtype=np.float32)
        for a in range(A):
            # rows h'*5+a get E[a,w,h'] in every column h*5+a
            S[np.ix_(hh * A + a, hh * A + a)] = E[a, w, :][:, None]
        sel[0:64, w, 0:64] = S
        sel[64:128, w, 64:128] = S
    sel = sel.astype(ml_dtypes.bfloat16)

    in_maps = []
    for k in range(N_CORES):
        shard = x[k * B_CORE : (k + 1) * B_CORE]          # (64, 500, 300)
        xtk = np.ascontiguousarray(
            shard.transpose(2, 0, 1).reshape(D, B_CORE * L)
        ).astype(ml_dtypes.bfloat16)
        in_maps.append({"xt": xtk, "wm": wm, "sel": sel})
    return in_maps


def _unshard(results):
    attn = np.empty((BATCH, A, L), dtype=np.float32)
    rep = np.empty((BATCH, A, H), dtype=np.float32)
    for k in range(N_CORES):
        attn[k * B_CORE : (k + 1) * B_CORE] = results[k]["attn_o"]
        r = results[k]["rep_o"].reshape(2, 64, PAIRS)[:, :C, :]  # [2, 50, 32]
        # r[p, h*5+a, i] -> rep[k*64 + 2*i + p, a, h]
        r = r.reshape(2, H, A, PAIRS).transpose(3, 0, 2, 1)      # [32, 2, 5, 10]
        rep[k * B_CORE : (k + 1) * B_CORE] = r.reshape(B_CORE, A, H)
    return attn, rep


def run_on_device(review_emb, asp_embed, asp_proj, trace=False, **kw):
    nc = _get_nc()
    in_maps = _prep_inputs(review_emb, asp_embed, asp_proj)
    res = run_bass_kernel_spmd(
        nc, in_maps, core_ids=list(range(N_CORES)), trace=trace, **kw
    )
    return res


def kernel(review_emb, asp_embed, asp_proj):
    res = run_on_device(review_emb, asp_embed, asp_proj, trace=False)
    return _unshard(res.results)


# revision 5
# speedup vs baseline: 1.3787x; 1.3787x over previous
"""Trainium2 Bass kernel for sparse-attention aspect pooling.

reference math (per batch row b):
    proj[a,l,h]  = sum_d x[l,d] * P[a,d,h]
    score[a,l]   = sum_{w,h} proj[a,l+w-1,h] * E[a,w,h]   (zero-padded window)
    attn[a,l]    = softmax_l(score)
    rep[a,h]     = sum_l attn[a,l] * proj[a,l,h]

Sharding: pure data parallel over batch (512 -> 64 per core x 8 cores).
Each core processes 2 batch rows per iteration:
  - proj via col-tiled matmuls: psum[0:64]=b0, psum[64:128]=b1, rows c=(h*5+a)
  - score via 3 shifted matmuls against a block-diagonal selector built from E;
    output rows replicated across h so pooling is a row-wise mult+reduce
  - softmax needs no max-subtraction (|score| < ~0.05 for this problem's scale)
"""

import numpy as np
import ml_dtypes

import concourse.bass as bass
import concourse.mybir as mybir
import concourse.tile as tile
from concourse import bacc
from concourse.bass_utils import run_bass_kernel_spmd

BF16 = mybir.dt.bfloat16
F32 = mybir.dt.float32

N_CORES = 8
BATCH = 512
B_CORE = BATCH // N_CORES      # 64
L = 500
D = 300
A = 5
H = 10
CTX = 3
C = A * H                      # 50 used rows, padded to 64 per batch row
PAIRS = B_CORE // 2            # 32 iterations, 2 batch rows each

_CACHE = {}


def _build():
    nc = bacc.Bacc(
        "TRN2", target_bir_lowering=False, debug=False, num_devices=N_CORES
    )
    xt_h = nc.dram_tensor("xt", [D, B_CORE * L], BF16, kind="ExternalInput")
    wm_h = nc.dram_tensor("wm", [D, 64], BF16, kind="ExternalInput")
    sel_h = nc.dram_tensor("sel", [128, CTX, 128], BF16, kind="ExternalInput")
    attn_h = nc.dram_tensor("attn_o", [B_CORE, A, L], F32, kind="ExternalOutput")
    rep_h = nc.dram_tensor("rep_o", [128, PAIRS], F32, kind="ExternalOutput")

    xt = xt_h.ap()
    wm = wm_h.ap()
    sel = sel_h.ap()
    attn_o = attn_h.ap()
    rep_o = rep_h.ap()

    from contextlib import ExitStack

    with tile.TileContext(nc) as tc, ExitStack() as ctx:
        singles = ctx.enter_context(tc.tile_pool(name="singles", bufs=1))
        xin = ctx.enter_context(tc.tile_pool(name="xin", bufs=3))
        psum = ctx.enter_context(tc.tile_pool(name="psum", bufs=3, space="PSUM"))
        work = ctx.enter_context(tc.tile_pool(name="work", bufs=4))
        small = ctx.enter_context(tc.tile_pool(name="small", bufs=4))

        wt = singles.tile([128, 3, 64], BF16)
        nc.sync.dma_start(out=wt[:, 0, :], in_=wm[0:128, :])
        nc.sync.dma_start(out=wt[:, 1, :], in_=wm[128:256, :])
        nc.sync.dma_start(out=wt[0:44, 2, :], in_=wm[256:300, :])
        selt = singles.tile([128, CTX, 128], BF16)
        nc.sync.dma_start(out=selt[:, :, :], in_=sel[:, :, :])
        repS = singles.tile([128, PAIRS], F32)

        kchunks = ((0, 128), (1, 128), (2, 44))

        for i in range(PAIRS):
            c0 = i * 2 * L
            xa = xin.tile([128, 2 * L], BF16, tag="xa")
            xb = xin.tile([128, 2 * L], BF16, tag="xb")
            xc = xin.tile([128, 2 * L], BF16, tag="xc")
            qc = nc.sync if i % 2 == 0 else nc.scalar
            nc.sync.dma_start(out=xa[:, :], in_=xt[0:128, c0 : c0 + 2 * L])
            nc.scalar.dma_start(out=xb[:, :], in_=xt[128:256, c0 : c0 + 2 * L])
            qc.dma_start(out=xc[0:44, :], in_=xt[256:300, c0 : c0 + 2 * L])
            xk_tiles = (xa, xb, xc)

            # proj: psum rows [0:64] = batch 2i, rows [64:128] = batch 2i+1
            pP = psum.tile([128, 512], F32, tag="pP")
            for half in (0, 1):
                for k, krows in kchunks:
                    xk = xk_tiles[k]
                    nc.tensor.matmul(
                        pP[64 * half : 64 * half + 64, 0:L],
                        wt[0:krows, k, :],
                        xk[0:krows, half * L : half * L + L],
                        start=(k == 0),
                        stop=(k == 2),
                    )

            sbP = work.tile([128, L], BF16, tag="sbP")
            nc.scalar.copy(out=sbP[:, :], in_=pP[:, 0:L])

            # score: 3 shifted selector matmuls accumulate into pS
            pS = psum.tile([128, 512], F32, tag="pS")
            nc.tensor.matmul(
                pS[:, 0:L], selt[:, 1, :], sbP[:, 0:L], start=True, stop=False
            )
            nc.tensor.matmul(
                pS[:, 1:L], selt[:, 0, :], sbP[:, 0 : L - 1], start=False, stop=False
            )
            nc.tensor.matmul(
                pS[:, 0 : L - 1], selt[:, 2, :], sbP[:, 1:L], start=False, stop=True
            )

            sbE = work.tile([128, L], BF16, tag="sbE")
            den = small.tile([128, 1], F32, tag="den")
            nc.scalar.activation(
                out=sbE[:, :],
                in_=pS[:, 0:L],
                func=mybir.ActivationFunctionType.Exp,
                accum_out=den[:, :],
            )
            invden = small.tile([128, 1], F32, tag="invden")
            nc.vector.reciprocal(out=invden[:, :], in_=den[:, :])

            # rep[:, i] = sum_l (sbE * invden) * sbP  — attn-weighted pooling
            trash = work.tile([128, L], BF16, tag="trash")
            nc.vector.scalar_tensor_tensor(
                out=trash[:, :],
                in0=sbE[:, :],
                scalar=invden[:, :],
                in1=sbP[:, :],
                op0=mybir.AluOpType.mult,
                op1=mybir.AluOpType.mult,
                accum_out=repS[:, i : i + 1],
            )

            attnS = work.tile([128, L], F32, tag="attnS")
            nc.vector.tensor_scalar_mul(attnS[:, :], sbE[:, :], invden[:, :])

            qa = nc.scalar if i % 2 == 0 else nc.sync
            qa.dma_start(out=attn_o[2 * i, :, :], in_=attnS[0:A, :])
            qa.dma_start(out=attn_o[2 * i + 1, :, :], in_=attnS[64 : 64 + A, :])

        nc.sync.dma_start(out=rep_o[:, :], in_=repS[:, :])

    nc.compile()
    return nc


def _get_nc():
    if "nc" not in _CACHE:
        _CACHE["nc"] = _build()
    return _CACHE["nc"]


def _prep_inputs(review_emb, asp_embed, asp_proj):
    """Host-side shard + layout prep. Returns in_maps for run_bass_kernel_spmd."""
    x = np.asarray(review_emb, dtype=np.float32)
    E = np.asarray(asp_embed, dtype=np.float32).reshape(A, CTX, H)
    P = np.asarray(asp_proj, dtype=np.float32)

    # W[d, h*5+a] = P[a,d,h], padded to 64 cols
    wm = np.zeros((D, 64), dtype=np.float32)
    wm[:, :C] = P.transpose(1, 2, 0).reshape(D, C)
    wm = wm.astype(ml_dtypes.bfloat16)

    # selector: S_w[(h'*5+a'), (h*5+a)] = delta(a'==a) * E[a,w,h']
    # block-diagonal duplicate for the two stacked batch rows
    sel = np.zeros((128, CTX, 128), dtype=np.float32)
    hh = np.arange(H)
    for w in range(CTX):
        S = np.zeros((64, 64), dtype=np.float32)
        for a in range(A):
            # rows h'*5+a get E[a,w,h'] in every column h*5+a
            S[np.ix_(hh * A + a, hh * A + a)] = E[a, w, :][:, None]
        sel[0:64, w, 0:64] = S
        sel[64:128, w, 64:128] = S
    sel = sel.astype(ml_dtypes.bfloat16)

    in_maps = []
    for k in range(N_CORES):
        shard = x[k * B_CORE : (k + 1) * B_CORE]          # (64, 500, 300)
        xtk = np.ascontiguousarray(
            shard.transpose(2, 0, 1).reshape(D, B_CORE * L)
        ).astype(ml_dtypes.bfloat16)
        in_maps.append({"xt": xtk, "wm": wm, "sel": sel})
    return in_maps


def _unshard(results):
    attn = np.empty((BATCH, A, L), dtype=np.float32)
    rep = np.empty((BATCH, A, H), dtype=np.float32)
    for k in range(N_CORES):
        attn[k * B_CORE : (k + 1) * B_CORE] = results[k]["attn_o"]
        r = results[k]["rep_o"].reshape(2, 64, PAIRS)[:, :C, :]  # [2, 50, 32]
        # r[p, h*5+a, i] -> rep[k*64 + 2*i + p, a, h]
        r = r.reshape(2, H, A, PAIRS).transpose(3, 0, 2, 1)      # [32, 2, 5, 10]
        rep[k * B_CORE : (k + 1) * B_CORE] = r.reshape(B_CORE, A, H)
    return attn, rep


def run_on_device(review_emb, asp_embed, asp_proj, trace=False, **kw):
    nc = _get_nc()
    in_maps = _prep_inputs(review_emb, asp_embed, asp_proj)
    res = run_bass_kernel_spmd(
        nc, in_maps, core_ids=list(range(N_CORES)), trace=trace, **kw
    )
    return res


def kernel(review_emb, asp_embed, asp_proj):
    res = run_on_device(review_emb, asp_embed, asp_proj, trace=False)
    return _unshard(res.results)
